# revision 21
# baseline (speedup 1.0000x reference)
"""Trainium2 Bass kernel for a dense transformer block (nn_Block_31387620999284).

Sharding: 8 cores = 4 batches x 2 parity groups. Core c handles batch b=c//2
and the query tokens with sequence parity d=c%2. Every core computes K/V for
its batch's full 2048-token sequence, so there is no cross-core communication.
To keep the instruction stream identical across cores (SPMD), odd-parity cores
receive the token sequence with each even/odd pair swapped so that "query
tokens" are always the even positions; the causal-diagonal mask (per-core
input data) absorbs the permutation.

v3 highlights over v2:
- FFN second matmul runs fp8 DoubleRow (w2 scaled by 2**13, relu output
  quantized to fp8); the b2 bias rides a GpSimd epilogue add.
- Global software pipeline: attention query-block 0 is interleaved with the
  tail of the QKV projections (keeps PE warm while the softmax EXP stream
  saturates the scalar engine), and out-proj/LN2/FFN1 of query half 0 are
  interleaved with attention query-block 1.
- w1/w2 are fully SBUF-resident: w1 prefetched at the end of the t=0 window,
  w2 during the attention tail, so the FFN phase issues no weight DMAs.
- LN2 statistics use ones-vector PE matmuls (per dp-group accumulation)
  instead of a DVE pairwise-add tree; squares moved from ACT to DVE.
"""

import sys

for _p in ("/opt/trn_rl_repo",):
    if _p not in sys.path:
        sys.path.append(_p)

import numpy as np
import ml_dtypes
from collections import deque
from contextlib import ExitStack

import concourse.bass as bass
import concourse.tile as tile
from concourse import bacc, mybir
from concourse.bass import ts
from concourse.bass_utils import run_bass_kernel_spmd


def _install_ntff_hook():
    """The container's antenv stub lacks axon_hooks; provide it so tracing
    (BASS_TRACE=1) works instead of crashing on import."""
    try:
        import antenv.axon_hooks  # noqa: F401
        return
    except ImportError:
        pass
    try:
        import types
        import antenv
        mod = types.ModuleType("antenv.axon_hooks")
        mod._hook = None
        mod.set_axon_ntff_profile_hook = lambda h: setattr(mod, "_hook", h)
        mod.get_axon_ntff_profile_hook = lambda: mod._hook
        sys.modules["antenv.axon_hooks"] = mod
        antenv.axon_hooks = mod
        try:
            from trn_agent_boot.trn_boot import _ntff_profile_via_ctypes
            mod._hook = _ntff_profile_via_ctypes("/opt/axon/libaxon_pjrt.so")
        except Exception:
            pass
    except Exception:
        pass


_install_ntff_hook()

P = 128
D = 1024
TKV = 2048
TQ = 1024
F = 4096
H = 16
HD = 64
DP = D // P    # 8
FP = F // P    # 32
CH = 512       # token chunk / matmul free dim
QB = 512       # attention query block
NQB = TQ // QB # 2
EPS = 1e-5

F32 = mybir.dt.float32
BF16 = mybir.dt.bfloat16
FP8 = mybir.dt.float8e4
AF = mybir.ActivationFunctionType
ALU = mybir.AluOpType
DR = mybir.MatmulPerfMode.DoubleRow

# power-of-2 fp8 weight scales (host multiplies weights by 2**K_*, the
# epilogues divide the PSUM result back down)
K_QKV = 12   # |w| <= 1/32 -> max 128
K_O = 12
K_1 = 12
K_2 = 13     # |w2| <= 1/64 -> max 128


import os
KPHASES = int(os.environ.get("KPHASES", "3"))


def build_nc():
    nc = bacc.Bacc("TRN2", target_bir_lowering=False, debug=False)

    xn = nc.dram_tensor("xn", [TKV, D], BF16, kind="ExternalInput").ap()
    xoT = nc.dram_tensor("xoT", [D, TQ], F32, kind="ExternalInput").ap()
    wq = nc.dram_tensor("wq", [D, D], FP8, kind="ExternalInput").ap()
    wk = nc.dram_tensor("wk", [D, D], FP8, kind="ExternalInput").ap()
    wv = nc.dram_tensor("wv", [D, D], FP8, kind="ExternalInput").ap()
    wo = nc.dram_tensor("wo", [D, D], FP8, kind="ExternalInput").ap()
    w1 = nc.dram_tensor("w1", [D, F], FP8, kind="ExternalInput").ap()
    w2 = nc.dram_tensor("w2", [F, D], FP8, kind="ExternalInput").ap()
    # bias columns: b2 8:16 | bq 16:24 | bk 24:32 | b1' 32:64
    biases = nc.dram_tensor("biases", [P, 64], F32, kind="ExternalInput").ap()
    bvr = nc.dram_tensor("bvr", [P, D], F32, kind="ExternalInput").ap()
    mk = nc.dram_tensor("mk", [P, 2, 64], BF16, kind="ExternalInput").ap()
    outT = nc.dram_tensor("outT", [D, TQ], F32, kind="ExternalOutput").ap()

    xn3 = xn.rearrange("(i p) d -> i p d", p=P)        # 16 tiles of 128 tokens
    xoT3 = xoT.rearrange("(o p) t -> p o t", p=P)
    out3 = outT.rearrange("(o p) t -> p o t", p=P)
    wq3 = wq.rearrange("(o p) m -> p o m", p=P)
    wk3 = wk.rearrange("(o p) m -> p o m", p=P)
    wv3 = wv.rearrange("(o p) m -> p o m", p=P)
    wo3 = wo.rearrange("(o p) m -> p o m", p=P)
    w13 = w1.rearrange("(o p) m -> p o m", p=P)
    w23 = w2.rearrange("(o p) m -> p o m", p=P)

    with tile.TileContext(nc) as tc, ExitStack() as ctx:
        consts = ctx.enter_context(tc.tile_pool(name="consts", bufs=1))
        bias_sb = consts.tile([P, 64], F32, name="bias_sb")
        nc.sync.dma_start(bias_sb[:], biases)
        ones_b16 = consts.tile([P, 1], BF16, name="ones_b16")
        nc.vector.memset(ones_b16[:], 1.0)
        ones_f32 = consts.tile([P, 1], F32, name="ones_f32")
        nc.vector.memset(ones_f32[:], 1.0)
        # warm the ACT function tables before the LN pipeline needs them (a
        # mid-phase ACT_TABLE_LOAD stalls the strict-FIFO scalar engine)
        wrm = consts.tile([1, 1], F32, name="wrm")
        nc.vector.memset(wrm[:], 1.0)
        wrm2 = consts.tile([1, 1], F32, name="wrm2")
        nc.scalar.activation(wrm2[:], wrm[:], AF.Sqrt)
        nc.scalar.activation(wrm2[:], wrm[:], AF.Exp)
        nc.scalar.activation(wrm2[:], wrm[:], AF.Square)
        nc.scalar.activation(wrm2[:], wrm[:], AF.Relu)

        # Long-lived right-side pools.
        sWX = ExitStack()
        mskp = sWX.enter_context(tc.tile_pool(name="mskp", bufs=1, side="right"))
        mask_sb = mskp.tile([P, 2, 64], BF16, name="mask_sb")
        nc.sync.dma_start(mask_sb[:], mk)
        wop = sWX.enter_context(tc.tile_pool(name="wop", bufs=1, side="right"))
        wo_sb = wop.tile([P, DP, D], FP8, name="wo_sb")
        xop = sWX.enter_context(tc.tile_pool(name="xop", bufs=2, side="right"))

        # Pools that must outlive phase 1 open BEFORE the phase-1 pools so
        # every memory side releases in LIFO order.
        sX2 = ExitStack()
        x2Ts = []
        h28s = []
        sATT = ExitStack()
        attp = sATT.enter_context(tc.tile_pool(name="attp", bufs=1, side="right"))
        attn8s = [attp.tile([P, DP, QB], FP8, name=f"attn8_{t}") for t in range(2)]
        weip = sATT.enter_context(tc.tile_pool(name="weip", bufs=3))
        smal = sATT.enter_context(tc.tile_pool(name="smal", bufs=2))
        l2q = sATT.enter_context(tc.tile_pool(name="l2q", bufs=3))
        l2s = sATT.enter_context(tc.tile_pool(name="l2s", bufs=1))
        if KPHASES >= 2:
            sPS0 = ExitStack()   # attention PSUM pools for the t=0 window
            psS = sPS0.enter_context(tc.tile_pool(name="psS", bufs=2, space="PSUM"))
            psAV = sPS0.enter_context(tc.tile_pool(name="psAV", bufs=2, space="PSUM"))

        # Persistent K/V/Q for attention, split per 1024-key half / query
        # block so interleaved producers and consumers never share a tile.
        sKVQ = ExitStack()
        kvqp = sKVQ.enter_context(tc.tile_pool(name="kvqp", bufs=1))
        KT_A = kvqp.tile([P, DP, TQ], BF16, name="KT_A")
        KT_B = kvqp.tile([P, DP, TQ], BF16, name="KT_B")
        V_A = kvqp.tile([P, 8, H, HD + 1], FP8, name="V_A")
        V_B = kvqp.tile([P, 8, H, HD + 1], FP8, name="V_B")
        QT0 = kvqp.tile([P, DP, QB], BF16, name="QT0")
        QT1 = kvqp.tile([P, DP, QB], BF16, name="QT1")
        KTs = (KT_A, KT_B)
        Vs = (V_A, V_B)
        QTs = (QT0, QT1)

        # ================= Phase 1 pools: LN1 + Q/K/V projections ===========
        p1 = ExitStack()
        xtp = p1.enter_context(tc.tile_pool(name="xtp", bufs=3))
        sqsp = p1.enter_context(tc.tile_pool(name="sqsp", bufs=1))
        lnsp = p1.enter_context(tc.tile_pool(name="lnsp", bufs=6))
        hnp = p1.enter_context(tc.tile_pool(name="hnp", bufs=3))
        hctp = p1.enter_context(tc.tile_pool(name="hctp", bufs=2))
        hc8p = p1.enter_context(tc.tile_pool(name="hc8p", bufs=2))
        q8p = p1.enter_context(tc.tile_pool(name="q8p", bufs=2))
        mmp = p1.enter_context(tc.tile_pool(name="mmp1", bufs=2, space="PSUM"))
        bvp = p1.enter_context(tc.tile_pool(name="bvp", bufs=1))
        wkvp = p1.enter_context(tc.tile_pool(name="wkvp", bufs=1))
        wqp = p1.enter_context(tc.tile_pool(name="wqp", bufs=1, side="right"))

        # bulk weight loads on the gpsimd DMA queue so they don't delay
        # the per-tile x streams on the sync queue
        wk_sb = wkvp.tile([P, DP, D], FP8, name="wk_sb")
        nc.gpsimd.dma_start(wk_sb[:], wk3)
        wv_sb = wkvp.tile([P, DP, D], FP8, name="wv_sb")
        nc.gpsimd.dma_start(wv_sb[:], wv3)
        wq_sb = wqp.tile([P, DP, D], FP8, name="wq_sb")
        nc.gpsimd.dma_start(wq_sb[:], wq3)
        nc.gpsimd.dma_start(wo_sb[:], wo3)
        bvr_sb = bvp.tile([P, D], F32, name="bvr_sb")
        nc.gpsimd.dma_start(bvr_sb[:], bvr)
        nc.vector.memset(V_A[:, :, :, HD:HD + 1], 1.0)
        nc.vector.memset(V_B[:, :, :, HD:HD + 1], 1.0)

        def ln_tile(i, hcT):
            """LN over one 128-token tile in token-major layout, then
            DMA-transpose into hcT[:, :, (i%4)*128 : ...]."""
            xt = xtp.tile([P, D], BF16, name="xt", tag="xt")
            nc.scalar.dma_start(xt[:], xn3[i])
            s1 = lnsp.tile([P, 1], F32, name="s1", tag="s1")
            nc.vector.tensor_reduce(s1[:], xt[:], axis=mybir.AxisListType.X,
                                    op=ALU.add)
            sqs = sqsp.tile([P, D], BF16, name="sqs", tag="sqs")
            s2 = lnsp.tile([P, 1], F32, name="s2", tag="s2")
            # (tensor_tensor_reduce crashes TRN2 hw; ACT Square+accum works)
            nc.scalar.activation(sqs[:], xt[:], AF.Square, accum_out=s2[:])
            negmu = lnsp.tile([P, 1], F32, name="negmu", tag="negmu")
            nc.vector.tensor_scalar_mul(negmu[:], s1[:], -1.0 / D)
            muu = lnsp.tile([P, 1], F32, name="muu", tag="muu")
            nc.vector.tensor_mul(muu[:], negmu[:], negmu[:])
            sdb = lnsp.tile([P, 1], F32, name="sdb", tag="sdb")
            nc.vector.tensor_scalar(sdb[:], muu[:], -1.0, EPS,
                                    op0=ALU.mult, op1=ALU.add)
            sd = lnsp.tile([P, 1], F32, name="sd", tag="sd")
            nc.scalar.activation(sd[:], s2[:], AF.Sqrt, bias=sdb[:],
                                 scale=1.0 / D)
            rstd = lnsp.tile([P, 1], F32, name="rstd", tag="rstd")
            nc.vector.reciprocal(rstd[:], sd[:])
            hN = hnp.tile([P, D], BF16, name="hN", tag="hN")
            nc.gpsimd.tensor_scalar(hN[:], xt[:], negmu[:], rstd[:],
                                    op0=ALU.add, op1=ALU.mult)
            j = i % 4
            nc.sync.dma_start_transpose(hcT[:, :, ts(j, P)], hN[:])

        hc8s = {}
        q8s = {}

        def make_chunk(c):
            """LN tiles + per-tile casts for KV chunk c; produces hc8 (and
            half of a q8 chunk)."""
            hcT = hctp.tile([P, DP, CH], BF16, name="hcT", tag="hcT")
            hc8 = hc8p.tile([P, DP, CH], FP8, name="hc8", tag="hc8")
            hc8s[c] = hc8
            qc, qh = divmod(c, 2)
            if qh == 0:
                q8s[qc] = q8p.tile([P, DP, CH], FP8, name="q8", tag="q8")
            q8 = q8s[qc]
            for j in range(4):
                ln_tile(c * 4 + j, hcT)
                nc.scalar.activation(hc8[:, :, ts(j, P)], hcT[:, :, ts(j, P)],
                                     AF.Copy)
                # even token positions of this tile -> q tokens
                nc.scalar.activation(
                    q8[:, :, qh * (CH // 2) + j * 64:qh * (CH // 2) + (j + 1) * 64],
                    hcT[:, :, j * P:(j + 1) * P:2], AF.Copy)

        def kproj(c, hp):
            hc8 = hc8s[c]
            ps = mmp.tile([P, CH], F32, name="psk", tag="mm1")
            for ks in range(4):
                nc.tensor.matmul(ps[:], wk_sb[:, 2 * ks:2 * ks + 2, ts(hp, P)],
                                 hc8[:, 2 * ks:2 * ks + 2, :],
                                 start=(ks == 0), stop=(ks == 3),
                                 perf_mode=DR)
            nc.vector.tensor_scalar(KTs[c // 2][:, hp, ts(c % 2, CH)], ps[:],
                                    2.0 ** -K_QKV,
                                    bias_sb[:, 24 + hp:25 + hp],
                                    op0=ALU.mult, op1=ALU.add)

        def vproj(c, st, dc):
            hc8 = hc8s[c]
            ps = mmp.tile([P, CH], F32, name="psv", tag="mm1")
            for ks in range(4):
                nc.tensor.matmul(ps[:], hc8[:, 2 * ks:2 * ks + 2, ts(st, P)],
                                 wv_sb[:, 2 * ks:2 * ks + 2, ts(dc, CH)],
                                 start=(ks == 0), stop=(ks == 3),
                                 perf_mode=DR)
            vdst = Vs[c // 2][:, (c % 2) * 4 + st, dc * 8:dc * 8 + 8, 0:HD]
            nc.vector.scalar_tensor_tensor(
                vdst,
                ps[:].rearrange("p (h d) -> p h d", h=8),
                2.0 ** -K_QKV,
                bvr_sb[:, ts(dc, CH)].rearrange("p (h d) -> p h d", h=8),
                op0=ALU.mult, op1=ALU.add)

        def qproj(qc, hp):
            q8 = q8s[qc]
            ps = mmp.tile([P, CH], F32, name="psq", tag="mm1")
            for ks in range(4):
                nc.tensor.matmul(ps[:], wq_sb[:, 2 * ks:2 * ks + 2, ts(hp, P)],
                                 q8[:, 2 * ks:2 * ks + 2, :],
                                 start=(ks == 0), stop=(ks == 3),
                                 perf_mode=DR)
            nc.vector.tensor_scalar(QTs[qc][:, hp, :], ps[:],
                                    2.0 ** -K_QKV,
                                    bias_sb[:, 16 + hp:17 + hp],
                                    op0=ALU.mult, op1=ALU.add)

        # ---- early dense QKV: chunks kv0, kv1 + qproj 0 ----
        make_chunk(0)
        for hp in range(DP):
            kproj(0, hp)
        make_chunk(1)
        for st in range(4):
            for dc in range(2):
                vproj(0, st, dc)
        for hp in range(DP):
            kproj(1, hp)
        for st in range(4):
            for dc in range(2):
                vproj(1, st, dc)
        for hp in range(DP):
            qproj(0, hp)
        make_chunk(2)

        if KPHASES < 1:
            # debug: LN pipeline only; dump hc8 chunks 0..1 as f32
            with tc.tile_pool(name="dbg", bufs=2) as dbg:
                for c in range(2):
                    for i in range(DP):
                        dt_ = dbg.tile([P, CH], F32, name="dt", tag="dt")
                        nc.vector.tensor_copy(dt_[:], hc8s[c][:, i, :])
                        nc.sync.dma_start(out3[:, i, ts(c, CH)], dt_[:])

        # fill queue of leftover phase-1 PE work, pumped between attention
        # m-steps so the PE stays warm while EXP saturates the scalar engine
        fill = deque()

        def pump(n=1):
            for _ in range(n):
                if fill:
                    fill.popleft()()

        if KPHASES >= 1:
            for hp in range(DP):
                fill.append(lambda hp=hp: kproj(2, hp))
            for st in range(2):
                for dc in range(2):
                    fill.append(lambda st=st, dc=dc: vproj(2, st, dc))
            fill.append(lambda: make_chunk(3))
            for st in range(2, 4):
                for dc in range(2):
                    fill.append(lambda st=st, dc=dc: vproj(2, st, dc))
            for hp in range(DP):
                fill.append(lambda hp=hp: kproj(3, hp))
            for st in range(4):
                for dc in range(2):
                    fill.append(lambda st=st, dc=dc: vproj(3, st, dc))
            for hp in range(DP):
                fill.append(lambda hp=hp: qproj(1, hp))

        # =================== attention machinery =============================
        scale = float(HD) ** -0.5

        if KPHASES >= 2:
            def scores_kt(t, hp, kt, wei8):
                """Scores + exp for one 128-key tile (both heads of pair hp)."""
                o = 64 * (kt - 8 * t) if kt >= 8 * t else 0
                op = 64 * ((kt & ~1) - 8 * t) if (kt & ~1) >= 8 * t else 0
                KT = KTs[kt // 8]
                ktl = kt % 8
                ps2 = psS.tile([P, 2, QB], F32, name="pss2", tag="pss2")
                for l in range(2):
                    pb = 64 * l
                    nc.tensor.matmul(ps2[:, l, o:],
                                     KT[pb:pb + 64, hp, ts(ktl, P)],
                                     QTs[t][pb:pb + 64, hp, o:],
                                     start=True, stop=True)
                nc.scalar.activation(wei8[:, kt & 1, :, o:], ps2[:, :, o:],
                                     AF.Exp, scale=scale)
                if o > op:
                    # odd diagonal tile: zero the columns its even partner
                    # covers but it does not
                    nc.vector.memset(wei8[:, 1, :, op:o], 0.0)
                if kt >= 8 * t:
                    nc.vector.tensor_mul(wei8[:, kt & 1, :, o:o + 64],
                                         wei8[:, kt & 1, :, o:o + 64],
                                         mask_sb[:])

            def attn_unit(t, hp, per_step=0):
                """One (query block, head pair) attention unit with software
                pipelining: AV for m-1 issues after scores for m, so the AV
                matmuls overlap the exp of the next key pair."""
                nkt = 8 * (t + 1)
                nm = nkt // 2
                pavs = [psAV.tile([P, QB], F32, name=f"pav{l}", tag="pav")
                        for l in range(2)]
                wei8s = {}
                for m in range(nm + 1):
                    if m < nm:
                        wei8s[m] = weip.tile([P, 2, 2, QB], FP8,
                                             name="wei8", tag="wei8")
                        scores_kt(t, hp, 2 * m, wei8s[m])
                        scores_kt(t, hp, 2 * m + 1, wei8s[m])
                    if m >= 1:
                        ma = m - 1
                        op = 64 * (2 * ma - 8 * t) if 2 * ma >= 8 * t else 0
                        wei8 = wei8s.pop(ma)
                        Vx = Vs[ma // 4]
                        mloc = ma % 4
                        for l in range(2):
                            nc.tensor.matmul(pavs[l][0:65, op:],
                                             Vx[:, 2 * mloc:2 * mloc + 2, 2 * hp + l, :],
                                             wei8[:, :, l, op:],
                                             start=(ma == 0), stop=(ma == nm - 1),
                                             perf_mode=DR)
                    pump(per_step)
                # denominator + attn8 write (immediate)
                for l in range(2):
                    pb = 64 * l
                    pav = pavs[l]
                    den = smal.tile([1, QB], F32, name="den", tag="den")
                    nc.vector.tensor_copy(den[:], pav[64:65, :])
                    rec = smal.tile([1, QB], F32, name="rec", tag="rec")
                    nc.vector.reciprocal_approx_fast(rec[:], den[:])
                    rec64 = smal.tile([64, QB], F32, name="rec64", tag="rec64")
                    nc.gpsimd.partition_broadcast(rec64[:], rec[:], channels=64)
                    nc.vector.tensor_mul(attn8s[t][pb:pb + 64, hp, :],
                                         pav[0:64, :], rec64[:])

            # ---- t=0 attention interleaved with the QKV tail ----
            for hp in range(DP):
                attn_unit(0, hp, per_step=1)
            pump(len(fill))  # drain leftovers

        if KPHASES < 2:
            # debug: dump KT (or QT with KDUMP=QT) and stop
            pump(len(fill))
            with tc.tile_pool(name="dbg", bufs=2) as dbg:
                if os.environ.get("KDUMP") == "QT":
                    for i in range(DP):
                        dt_ = dbg.tile([P, QB], F32, name="dt", tag="dt")
                        nc.vector.tensor_copy(dt_[:], QTs[0][:, i, :])
                        nc.sync.dma_start(out3[:, i, 0:QB], dt_[:])
                        dt2 = dbg.tile([P, QB], F32, name="dt2", tag="dt2")
                        nc.vector.tensor_copy(dt2[:], QTs[1][:, i, :])
                        nc.sync.dma_start(out3[:, i, QB:], dt2[:])
                else:
                    for i in range(DP):
                        dt_ = dbg.tile([P, TQ], F32, name="dt", tag="dt")
                        nc.vector.tensor_copy(dt_[:], KTs[0][:, i, :])
                        nc.sync.dma_start(out3[:, i, :], dt_[:])
            p1.close()
            sKVQ.close()
            sATT.close()
            sX2.close()
            sWX.close()

        if KPHASES >= 2:
            # phase-1 pools retire; prefetch w1 (needed mid-t=1 by FFN1 qc0)
            p1.close()
            sPS0.close()
            x2p = sX2.enter_context(tc.tile_pool(name="x2p", bufs=1, side="right"))
            x2Ts.extend(x2p.tile([P, DP, QB], BF16, name=f"x2T{t}") for t in range(2))
            h28s.extend(x2p.tile([P, DP, QB], FP8, name=f"h28_{t}") for t in range(2))
            ffw = ExitStack()
            w1p = ffw.enter_context(tc.tile_pool(name="w1p", bufs=1, side="right"))
            w1_sb = w1p.tile([P, DP, F], FP8, name="w1_sb")
            nc.gpsimd.dma_start(w1_sb[:, :, 0:F // 2], w13[:, :, 0:F // 2])
            nc.gpsimd.dma_start(w1_sb[:, :, F // 2:], w13[:, :, F // 2:])
            rtp = ffw.enter_context(tc.tile_pool(name="rtp", bufs=1, side="right"))
            rT0 = rtp.tile([P, FP, CH], FP8, name="rT", tag="rT")

            psF = sATT.enter_context(tc.tile_pool(name="psF", bufs=2, space="PSUM"))
            sPS1 = ExitStack()   # attention PSUM pools for the t=1 window
            psS = sPS1.enter_context(tc.tile_pool(name="psS2", bufs=2, space="PSUM"))
            psAV = sPS1.enter_context(tc.tile_pool(name="psAV2", bufs=2, space="PSUM"))

            def wo_group(t, i):
                """One out-projection output tile + residual epilogue -> x2T."""
                ps = psF.tile([P, CH], F32, name="psf", tag="psf")
                for ks in range(4):
                    nc.tensor.matmul(ps[:], wo_sb[:, 2 * ks:2 * ks + 2, ts(i, P)],
                                     attn8s[t][:, 2 * ks:2 * ks + 2, :],
                                     start=(ks == 0), stop=(ks == 3),
                                     perf_mode=DR)
                xo = xop.tile([P, CH], F32, name="xo", tag="xo")
                nc.sync.dma_start(xo[:], xoT3[:, i, ts(t, CH)])
                nc.vector.scalar_tensor_tensor(x2Ts[t][:, i, :], ps[:],
                                               2.0 ** -K_O, xo[:],
                                               op0=ALU.mult, op1=ALU.add)

            def ln2_half(qc):
                """LN2 for query half qc -> h28s[qc]. Stats via ones-matmul
                accumulation on the PE; squares on DVE."""
                x2T = x2Ts[qc]
                ps_s1 = psF.tile([P, CH], F32, name="psf", tag="psf")
                for i in range(DP):
                    nc.tensor.matmul(ps_s1[0:1, :], ones_b16[:], x2T[:, i, :],
                                     start=(i == 0), stop=(i == DP - 1))
                negmu2 = l2s.tile([1, CH], F32, name="negmu2", tag="negmu2")
                nc.vector.tensor_scalar_mul(negmu2[:], ps_s1[0:1, :], -1.0 / D)
                ps_s2 = psF.tile([P, CH], F32, name="psf", tag="psf")
                for i in range(DP):
                    sq = l2q.tile([P, CH], BF16, name="sq2", tag="sq2")
                    nc.vector.tensor_mul(sq[:], x2T[:, i, :], x2T[:, i, :])
                    nc.tensor.matmul(ps_s2[0:1, :], ones_b16[:], sq[:],
                                     start=(i == 0), stop=(i == DP - 1))
                muu2 = l2s.tile([1, CH], F32, name="muu2", tag="muu2")
                nc.vector.tensor_mul(muu2[:], negmu2[:], negmu2[:])
                sdb2 = l2s.tile([1, CH], F32, name="sdb2", tag="sdb2")
                nc.vector.tensor_scalar(sdb2[:], muu2[:], -1.0, EPS,
                                        op0=ALU.mult, op1=ALU.add)
                var2 = l2s.tile([1, CH], F32, name="var2", tag="var2")
                nc.vector.scalar_tensor_tensor(var2[:], ps_s2[0:1, :],
                                               1.0 / D, sdb2[:],
                                               op0=ALU.mult, op1=ALU.add)
                sd2 = l2s.tile([1, CH], F32, name="sd2", tag="sd2")
                nc.scalar.activation(sd2[:], var2[:], AF.Sqrt)
                rstd2 = l2s.tile([1, CH], F32, name="rstd2", tag="rstd2")
                nc.vector.reciprocal_approx_fast(rstd2[:], sd2[:])
                nmr2 = l2s.tile([1, CH], BF16, name="nmr2", tag="nmr2")
                nc.vector.tensor_mul(nmr2[:], negmu2[:], rstd2[:])
                rstd2b = l2s.tile([P, CH], F32, name="rstd2b", tag="rstd2b")
                nc.gpsimd.partition_broadcast(rstd2b[:], rstd2[:], channels=P)
                nmr2b = l2s.tile([P, CH], BF16, name="nmr2b", tag="nmr2b")
                nc.gpsimd.partition_broadcast(nmr2b[:], nmr2[:], channels=P)
                for i in range(DP):
                    tmp = l2q.tile([P, CH], BF16, name="h2t", tag="h2t")
                    nc.gpsimd.tensor_mul(tmp[:], x2Ts[qc][:, i, :], rstd2b[:])
                    nc.vector.tensor_add(h28s[qc][:, i, :], tmp[:], nmr2b[:])

            def ffn1_group(qc, f, rT):
                h28 = h28s[qc]
                ps = psF.tile([P, CH], F32, name="psf", tag="psf")
                for ks in range(4):
                    nc.tensor.matmul(ps[:], w1_sb[:, 2 * ks:2 * ks + 2, ts(f, P)],
                                     h28[:, 2 * ks:2 * ks + 2, :],
                                     start=(ks == 0), stop=(ks == 3),
                                     perf_mode=DR)
                nc.scalar.activation(rT[:, f, :], ps[:],
                                     AF.Relu,
                                     bias=bias_sb[:, 32 + f:33 + f],
                                     scale=2.0 ** -K_1)

            # ---- t=1 attention interleaved with wo(0), LN2(0), FFN1 qc0 ----
            for i in range(DP):
                fill.append(lambda i=i: wo_group(0, i))
            fill.append(lambda: ln2_half(0))
            for f in range(FP):
                fill.append(lambda f=f: ffn1_group(0, f, rT0))

            for hp in range(DP):
                attn_unit(1, hp, per_step=1)
            pump(len(fill))
            sKVQ.close()
            sPS1.close()

        if KPHASES == 2:
            # debug: dump x2T (or attn8 with KDUMP=ATT) and stop
            with tc.tile_pool(name="dbg", bufs=2) as dbg:
                for t in range(2):
                    dsrc = attn8s[t] if os.environ.get("KDUMP") == "ATT" else x2Ts[t]
                    for i in range(DP):
                        dt_ = dbg.tile([P, QB], F32, name="dt", tag="dt")
                        nc.vector.tensor_copy(dt_[:], dsrc[:, i, :])
                        nc.sync.dma_start(out3[:, i, ts(t, CH)], dt_[:])
            ffw.close()
            sX2.close()
            sATT.close()
            sWX.close()

        if KPHASES >= 3:
            # ============== tail: wo(1), LN2(1), FFN2 qc0, FFN1/2 qc1 =======
            # attention PSUM pools have retired so FFN2 gets banks; w2 now
            w2p = ffw.enter_context(tc.tile_pool(name="w2p", bufs=1, side="right"))
            w2_sb = w2p.tile([P, FP, D], FP8, name="w2_sb")
            nc.gpsimd.dma_start(w2_sb[:, :, 0:D // 2], w23[:, :, 0:D // 2])
            nc.gpsimd.dma_start(w2_sb[:, :, D // 2:], w23[:, :, D // 2:])

            with tc.tile_pool(name="psO", bufs=2, space="PSUM") as psO, \
                 tc.tile_pool(name="top", bufs=4) as top:

                def ffn2_group(qc, i, rT):
                    ps2 = psO.tile([P, CH], F32, name="ps2", tag="ps2")
                    for f2 in range(FP // 2):
                        nc.tensor.matmul(ps2[:],
                                         w2_sb[:, 2 * f2:2 * f2 + 2, ts(i, P)],
                                         rT[:, 2 * f2:2 * f2 + 2, :],
                                         start=(f2 == 0), stop=(f2 == FP // 2 - 1),
                                         perf_mode=DR)
                    ot = top.tile([P, CH], F32, name="ot", tag="ot")
                    nc.vector.scalar_tensor_tensor(
                        ot[:], ps2[:], 2.0 ** -K_2,
                        x2Ts[qc][:, i, :], op0=ALU.mult, op1=ALU.add)
                    ot2 = top.tile([P, CH], F32, name="ot2", tag="ot2")
                    nc.gpsimd.tensor_scalar_add(ot2[:], ot[:],
                                                bias_sb[:, 8 + i:9 + i])
                    nc.sync.dma_start(out3[:, i, ts(qc, CH)], ot2[:])

                for i in range(DP):
                    wo_group(1, i)
                ln2_half(1)
                # FFN2 qc0 overlaps LN2(1)'s vector work; then the single rT
                # slot rotates to qc1 (alloc waits on the qc0 reads)
                for i in range(DP):
                    ffn2_group(0, i, rT0)
                rT1 = rtp.tile([P, FP, CH], FP8, name="rT", tag="rT")
                for f in range(FP):
                    ffn1_group(1, f, rT1)
                for i in range(DP):
                    ffn2_group(1, i, rT1)
            ffw.close()
            sX2.close()
            sATT.close()
            sWX.close()

    nc.compile()
    return nc


def _fp8_scale(w, k):
    """Scale w by 2**k and cast to fp8e4m3 (max-normal 240), asserting range."""
    s = np.asarray(w, np.float32) * (2.0 ** k)
    assert np.abs(s).max() < 239.0, f"fp8 overflow: {np.abs(s).max()}"
    return np.ascontiguousarray(s.astype(ml_dtypes.float8_e4m3))


def prepare_inputs(x, wq, wk, wv, wo, bo, w1, b1, w2, b2,
                   g_ln1, b_ln1, g_ln2, b_ln2):
    """Host-side sharding/prep. Returns list of 8 per-core input dicts."""
    f32 = np.float32
    bf = ml_dtypes.bfloat16
    x = np.asarray(x, f32)
    g1 = np.asarray(g_ln1, f32)
    b1n = np.asarray(b_ln1, f32)
    g2 = np.asarray(g_ln2, f32)
    b2n = np.asarray(b_ln2, f32)

    wq_e = _fp8_scale(g1[:, None] * np.asarray(wq, f32), K_QKV)
    wk_e = _fp8_scale(g1[:, None] * np.asarray(wk, f32), K_QKV)
    wv_e = _fp8_scale(g1[:, None] * np.asarray(wv, f32), K_QKV)
    wo_e = _fp8_scale(np.asarray(wo, f32), K_O)
    w1_e = _fp8_scale(g2[:, None] * np.asarray(w1, f32), K_1)
    w2_e = _fp8_scale(np.asarray(w2, f32), K_2)

    bq = b1n @ np.asarray(wq, f32)
    bk = b1n @ np.asarray(wk, f32)
    bv = b1n @ np.asarray(wv, f32)
    b1p = np.asarray(b1, f32) + b2n @ np.asarray(w1, f32)
    bo_f = np.asarray(bo, f32)
    b2_f = np.asarray(b2, f32)

    def pcol(v, n):  # [n*128] -> [128, n] partition-major
        return np.ascontiguousarray(np.asarray(v, f32).reshape(n, P).T)

    biases = np.zeros((P, 64), f32)
    biases[:, 8:16] = pcol(b2_f, 8)
    biases[:, 16:24] = pcol(bq, 8)
    biases[:, 24:32] = pcol(bk, 8)
    biases[:, 32:64] = pcol(b1p, 32)
    bvr = np.ascontiguousarray(np.broadcast_to(bv[None, :], (P, D)))

    # per-parity token permutation: queries always land on even positions
    perms = {}
    for d in (0, 1):
        perm = np.arange(TKV)
        if d == 1:
            perm = perm.reshape(-1, 2)[:, ::-1].reshape(-1)
        perms[d] = perm

    # diagonal-tile causal masks: key at tile position p (original token
    # perm[base+p]) may be attended by query column r (original token
    # perm[base+2r] = base + 2r + d) iff perm[base+p] <= base + 2r + d.
    masks = {}
    for d in (0, 1):
        pp = perms[d][:P]               # relative original positions
        r = np.arange(64)[None, :]
        m = (pp[:, None] <= (2 * r + d)).astype(bf)
        masks[d] = np.ascontiguousarray(np.broadcast_to(m[:, None, :], (P, 2, 64)))

    in_maps = []
    for c in range(8):
        b, d = divmod(c, 2)
        xo = x[b, d::2].T + bo_f[:, None]
        in_maps.append(dict(
            xn=np.ascontiguousarray(x[b][perms[d]].astype(bf)),
            xoT=np.ascontiguousarray(xo),
            wq=wq_e, wk=wk_e, wv=wv_e, wo=wo_e, w1=w1_e, w2=w2_e,
            biases=biases, bvr=bvr, mk=masks[d],
        ))
    return in_maps


_NC = None
LAST_RESULTS = None


def kernel(**inputs):
    global _NC, LAST_RESULTS
    in_maps = prepare_inputs(**inputs)
    if _NC is None:
        _NC = build_nc()
    res = run_bass_kernel_spmd(_NC, in_maps, core_ids=list(range(8)))
    LAST_RESULTS = res
    out = np.empty((4, TKV, D), np.float32)
    for c in range(8):
        b, d = divmod(c, 2)
        out[b, d::2, :] = res.results[c]["outT"].T
    return out


if __name__ == "__main__":
    z = np.load("/root/problem/ref_cache.npz")
    inputs = {k: z[k] for k in z.files if k != "out"}
    out = kernel(**inputs)
    ref = z["out"]
    err = np.abs(out - ref)
    print("abs max err:", err.max(), "scale-rel:", err.max() / np.abs(ref).max())


# revision 30
# speedup vs baseline: 1.2894x; 1.2894x over previous
"""Trainium2 Bass kernel for a dense transformer block (nn_Block_31387620999284).

Sharding: 8 cores = 4 batches x 2 parity groups. Core c handles batch b=c//2
and the query tokens with sequence parity d=c%2. Every core computes K/V for
its batch's full 2048-token sequence, so there is no cross-core communication.
To keep the instruction stream identical across cores (SPMD), odd-parity cores
receive the token sequence with each even/odd pair swapped so that "query
tokens" are always the even positions; the causal-diagonal mask (per-core
input data) absorbs the permutation.

v3 highlights over v2:
- FFN second matmul runs fp8 DoubleRow (w2 scaled by 2**13, relu output
  quantized to fp8); the b2 bias rides a GpSimd epilogue add.
- Global software pipeline: attention query-block 0 is interleaved with the
  tail of the QKV projections (keeps PE warm while the softmax EXP stream
  saturates the scalar engine), and out-proj/LN2/FFN1 of query half 0 are
  interleaved with attention query-block 1.
- w1/w2 are fully SBUF-resident: w1 prefetched at the end of the t=0 window,
  w2 during the attention tail, so the FFN phase issues no weight DMAs.
- LN2 statistics use ones-vector PE matmuls (per dp-group accumulation)
  instead of a DVE pairwise-add tree; squares moved from ACT to DVE.
"""

import sys

for _p in ("/opt/trn_rl_repo",):
    if _p not in sys.path:
        sys.path.append(_p)

import numpy as np
import ml_dtypes
from collections import deque
from contextlib import ExitStack

import concourse.bass as bass
import concourse.tile as tile
from concourse import bacc, mybir
from concourse.bass import ts
from concourse.bass_utils import run_bass_kernel_spmd


def _install_ntff_hook():
    """The container's antenv stub lacks axon_hooks; provide it so tracing
    (BASS_TRACE=1) works instead of crashing on import."""
    try:
        import antenv.axon_hooks  # noqa: F401
        return
    except ImportError:
        pass
    try:
        import types
        import antenv
        mod = types.ModuleType("antenv.axon_hooks")
        mod._hook = None
        mod.set_axon_ntff_profile_hook = lambda h: setattr(mod, "_hook", h)
        mod.get_axon_ntff_profile_hook = lambda: mod._hook
        sys.modules["antenv.axon_hooks"] = mod
        antenv.axon_hooks = mod
        try:
            from trn_agent_boot.trn_boot import _ntff_profile_via_ctypes
            mod._hook = _ntff_profile_via_ctypes("/opt/axon/libaxon_pjrt.so")
        except Exception:
            pass
    except Exception:
        pass


_install_ntff_hook()

P = 128
D = 1024
TKV = 2048
TQ = 1024
F = 4096
H = 16
HD = 64
DP = D // P    # 8
FP = F // P    # 32
CH = 512       # token chunk / matmul free dim
QB = 512       # attention query block
NQB = TQ // QB # 2
EPS = 1e-5

F32 = mybir.dt.float32
BF16 = mybir.dt.bfloat16
FP8 = mybir.dt.float8e4
AF = mybir.ActivationFunctionType
ALU = mybir.AluOpType
DR = mybir.MatmulPerfMode.DoubleRow

# power-of-2 fp8 weight scales (host multiplies weights by 2**K_*, the
# epilogues divide the PSUM result back down)
K_QKV = 12   # |w| <= 1/32 -> max 128
K_O = 12
K_1 = 12
K_2 = 13     # |w2| <= 1/64 -> max 128


import os
KPHASES = int(os.environ.get("KPHASES", "3"))


def build_nc():
    nc = bacc.Bacc("TRN2", target_bir_lowering=False, debug=False)

    xn = nc.dram_tensor("xn", [TKV, D], BF16, kind="ExternalInput").ap()
    xoT = nc.dram_tensor("xoT", [D, TQ], F32, kind="ExternalInput").ap()
    wq = nc.dram_tensor("wq", [D, D], FP8, kind="ExternalInput").ap()
    wk = nc.dram_tensor("wk", [D, D], FP8, kind="ExternalInput").ap()
    wv = nc.dram_tensor("wv", [D, D], FP8, kind="ExternalInput").ap()
    wo = nc.dram_tensor("wo", [D, D], FP8, kind="ExternalInput").ap()
    w1 = nc.dram_tensor("w1", [D, F], FP8, kind="ExternalInput").ap()
    w2 = nc.dram_tensor("w2", [F, D], FP8, kind="ExternalInput").ap()
    # bias columns: b2 8:16 | bq 16:24 | bk 24:32 | b1' 32:64
    biases = nc.dram_tensor("biases", [P, 64], F32, kind="ExternalInput").ap()
    b2r = nc.dram_tensor("b2r", [1, D], BF16, kind="ExternalInput").ap()
    bvr = nc.dram_tensor("bvr", [P, D], F32, kind="ExternalInput").ap()
    mk = nc.dram_tensor("mk", [P, 2, 64], BF16, kind="ExternalInput").ap()
    outT = nc.dram_tensor("outT", [D, TQ], F32, kind="ExternalOutput").ap()

    xn3 = xn.rearrange("(i p) d -> i p d", p=P)        # 16 tiles of 128 tokens
    xoT3 = xoT.rearrange("(o p) t -> p o t", p=P)
    out3 = outT.rearrange("(o p) t -> p o t", p=P)
    wq3 = wq.rearrange("(o p) m -> p o m", p=P)
    wk3 = wk.rearrange("(o p) m -> p o m", p=P)
    wv3 = wv.rearrange("(o p) m -> p o m", p=P)
    wo3 = wo.rearrange("(o p) m -> p o m", p=P)
    w13 = w1.rearrange("(o p) m -> p o m", p=P)
    w23 = w2.rearrange("(o p) m -> p o m", p=P)

    with tile.TileContext(nc) as tc, ExitStack() as ctx:
        consts = ctx.enter_context(tc.tile_pool(name="consts", bufs=1))
        bias_sb = consts.tile([P, 64], F32, name="bias_sb")
        nc.sync.dma_start(bias_sb[:], biases)
        ones_b16 = consts.tile([P, 1], BF16, name="ones_b16")
        nc.vector.memset(ones_b16[:], 1.0)
        onesq = consts.tile([1, CH], BF16, name="onesq")
        nc.vector.memset(onesq[:], 1.0)
        b2r_sb = consts.tile([1, D], BF16, name="b2r_sb")
        nc.sync.dma_start(b2r_sb[:], b2r)
        # warm the ACT function tables before the LN pipeline needs them (a
        # mid-phase ACT_TABLE_LOAD stalls the strict-FIFO scalar engine)
        wrm = consts.tile([1, 1], F32, name="wrm")
        nc.vector.memset(wrm[:], 1.0)
        wrm2 = consts.tile([1, 1], F32, name="wrm2")
        nc.scalar.activation(wrm2[:], wrm[:], AF.Sqrt)
        nc.scalar.activation(wrm2[:], wrm[:], AF.Exp)
        nc.scalar.activation(wrm2[:], wrm[:], AF.Square)
        nc.scalar.activation(wrm2[:], wrm[:], AF.Relu)

        # Long-lived right-side pools.
        sWX = ExitStack()
        mskp = sWX.enter_context(tc.tile_pool(name="mskp", bufs=1, side="right"))
        mask_sb = mskp.tile([P, 2, 64], BF16, name="mask_sb")
        nc.sync.dma_start(mask_sb[:], mk)
        wop = sWX.enter_context(tc.tile_pool(name="wop", bufs=1, side="right"))
        wo_sb = wop.tile([P, DP, D], FP8, name="wo_sb")
        xop = sWX.enter_context(tc.tile_pool(name="xop", bufs=2, side="right"))

        # Pools that must outlive phase 1 open BEFORE the phase-1 pools so
        # every memory side releases in LIFO order.
        sX2 = ExitStack()
        x2Ts = []
        h28s = []
        sATT = ExitStack()
        attp = sATT.enter_context(tc.tile_pool(name="attp", bufs=1, side="right"))
        attn8s = [attp.tile([P, DP, QB], FP8, name=f"attn8_{t}") for t in range(2)]
        weip = sATT.enter_context(tc.tile_pool(name="weip", bufs=3))
        smal = sATT.enter_context(tc.tile_pool(name="smal", bufs=2))
        l2q = sATT.enter_context(tc.tile_pool(name="l2q", bufs=3))
        l2s = sATT.enter_context(tc.tile_pool(name="l2s", bufs=1))
        if KPHASES >= 2:
            sPS0 = ExitStack()   # attention PSUM pools for the t=0 window
            psS = sPS0.enter_context(tc.tile_pool(name="psS", bufs=2, space="PSUM"))
            psAV = sPS0.enter_context(tc.tile_pool(name="psAV", bufs=2, space="PSUM"))

        # Persistent K/V/Q for attention, split per 1024-key half / query
        # block so interleaved producers and consumers never share a tile.
        sKVQ = ExitStack()
        kvqp = sKVQ.enter_context(tc.tile_pool(name="kvqp", bufs=1))
        KT_A = kvqp.tile([P, DP, TQ], BF16, name="KT_A")
        KT_B = kvqp.tile([P, DP, TQ], BF16, name="KT_B")
        V_A = kvqp.tile([P, 8, H, HD + 1], FP8, name="V_A")
        V_B = kvqp.tile([P, 8, H, HD + 1], FP8, name="V_B")
        QT0 = kvqp.tile([P, DP, QB], BF16, name="QT0")
        QT1 = kvqp.tile([P, DP, QB], BF16, name="QT1")
        KTs = (KT_A, KT_B)
        Vs = (V_A, V_B)
        QTs = (QT0, QT1)

        # ================= Phase 1 pools: LN1 + Q/K/V projections ===========
        p1 = ExitStack()
        xtp = p1.enter_context(tc.tile_pool(name="xtp", bufs=3))
        sqsp = p1.enter_context(tc.tile_pool(name="sqsp", bufs=1))
        lnsp = p1.enter_context(tc.tile_pool(name="lnsp", bufs=6))
        hnp = p1.enter_context(tc.tile_pool(name="hnp", bufs=3))
        hctp = p1.enter_context(tc.tile_pool(name="hctp", bufs=2))
        hc8p = p1.enter_context(tc.tile_pool(name="hc8p", bufs=2))
        q8p = p1.enter_context(tc.tile_pool(name="q8p", bufs=2))
        mmp = p1.enter_context(tc.tile_pool(name="mmp1", bufs=2, space="PSUM"))
        bvp = p1.enter_context(tc.tile_pool(name="bvp", bufs=1))
        wkvp = p1.enter_context(tc.tile_pool(name="wkvp", bufs=1))
        wqp = p1.enter_context(tc.tile_pool(name="wqp", bufs=1, side="right"))

        # bulk weight loads on the gpsimd DMA queue so they don't delay
        # the per-tile x streams on the sync queue
        wk_sb = wkvp.tile([P, DP, D], FP8, name="wk_sb")
        nc.gpsimd.dma_start(wk_sb[:], wk3)
        wv_sb = wkvp.tile([P, DP, D], FP8, name="wv_sb")
        nc.gpsimd.dma_start(wv_sb[:], wv3)
        wq_sb = wqp.tile([P, DP, D], FP8, name="wq_sb")
        nc.gpsimd.dma_start(wq_sb[:], wq3)
        nc.gpsimd.dma_start(wo_sb[:], wo3)
        bvr_sb = bvp.tile([P, D], F32, name="bvr_sb")
        nc.gpsimd.dma_start(bvr_sb[:], bvr)
        nc.vector.memset(V_A[:, :, :, HD:HD + 1], 1.0)
        nc.vector.memset(V_B[:, :, :, HD:HD + 1], 1.0)

        def ln_tile(i, hcT):
            """LN over one 128-token tile in token-major layout, then
            DMA-transpose into hcT[:, :, (i%4)*128 : ...]."""
            xt = xtp.tile([P, D], BF16, name="xt", tag="xt")
            nc.scalar.dma_start(xt[:], xn3[i])
            s1 = lnsp.tile([P, 1], F32, name="s1", tag="s1")
            nc.vector.tensor_reduce(s1[:], xt[:], axis=mybir.AxisListType.X,
                                    op=ALU.add)
            sqs = sqsp.tile([P, D], BF16, name="sqs", tag="sqs")
            s2 = lnsp.tile([P, 1], F32, name="s2", tag="s2")
            # (tensor_tensor_reduce crashes TRN2 hw; ACT Square+accum works)
            nc.scalar.activation(sqs[:], xt[:], AF.Square, accum_out=s2[:])
            negmu = lnsp.tile([P, 1], F32, name="negmu", tag="negmu")
            nc.vector.tensor_scalar_mul(negmu[:], s1[:], -1.0 / D)
            muu = lnsp.tile([P, 1], F32, name="muu", tag="muu")
            nc.vector.tensor_mul(muu[:], negmu[:], negmu[:])
            sdb = lnsp.tile([P, 1], F32, name="sdb", tag="sdb")
            nc.vector.tensor_scalar(sdb[:], muu[:], -1.0, EPS,
                                    op0=ALU.mult, op1=ALU.add)
            sd = lnsp.tile([P, 1], F32, name="sd", tag="sd")
            nc.scalar.activation(sd[:], s2[:], AF.Sqrt, bias=sdb[:],
                                 scale=1.0 / D)
            rstd = lnsp.tile([P, 1], F32, name="rstd", tag="rstd")
            nc.vector.reciprocal(rstd[:], sd[:])
            hN = hnp.tile([P, D], BF16, name="hN", tag="hN")
            nc.gpsimd.tensor_scalar(hN[:], xt[:], negmu[:], rstd[:],
                                    op0=ALU.add, op1=ALU.mult)
            j = i % 4
            nc.sync.dma_start_transpose(hcT[:, :, ts(j, P)], hN[:])

        hc8s = {}
        q8s = {}
        _lnq = deque()   # deferred per-tile casts (two-stage LN pipeline)

        def ln_casts():
            if _lnq:
                _lnq.popleft()()

        def make_chunk(c):
            """LN tiles + per-tile casts for KV chunk c; produces hc8 (and
            half of a q8 chunk). The fp8 casts are issued one tile BEHIND the
            stats so the in-order ACT queue isn't head-of-line blocked on the
            SBUF transpose of the current tile."""
            hcT = hctp.tile([P, DP, CH], BF16, name="hcT", tag="hcT")
            hc8 = hc8p.tile([P, DP, CH], FP8, name="hc8", tag="hc8")
            hc8s[c] = hc8
            qc, qh = divmod(c, 2)
            if qh == 0:
                q8s[qc] = q8p.tile([P, DP, CH], FP8, name="q8", tag="q8")
            q8 = q8s[qc]
            for j in range(4):
                ln_tile(c * 4 + j, hcT)

                def casts(j=j, hcT=hcT, hc8=hc8, q8=q8, qh=qh):
                    nc.scalar.activation(hc8[:, :, ts(j, P)],
                                         hcT[:, :, ts(j, P)], AF.Copy)
                    # even token positions of this tile -> q tokens
                    nc.scalar.activation(
                        q8[:, :, qh * (CH // 2) + j * 64:qh * (CH // 2) + (j + 1) * 64],
                        hcT[:, :, j * P:(j + 1) * P:2], AF.Copy)
                _lnq.append(casts)
                if len(_lnq) > 1:
                    ln_casts()

        def finish_chunk():
            while _lnq:
                ln_casts()

        def kproj(c, hp):
            hc8 = hc8s[c]
            ps = mmp.tile([P, CH], F32, name="psk", tag="mm1")
            for ks in range(4):
                nc.tensor.matmul(ps[:], wk_sb[:, 2 * ks:2 * ks + 2, ts(hp, P)],
                                 hc8[:, 2 * ks:2 * ks + 2, :],
                                 start=(ks == 0), stop=(ks == 3),
                                 perf_mode=DR)
            nc.vector.tensor_scalar(KTs[c // 2][:, hp, ts(c % 2, CH)], ps[:],
                                    2.0 ** -K_QKV,
                                    bias_sb[:, 24 + hp:25 + hp],
                                    op0=ALU.mult, op1=ALU.add)

        def vproj(c, st, dc):
            hc8 = hc8s[c]
            ps = mmp.tile([P, CH], F32, name="psv", tag="mm1")
            for ks in range(4):
                nc.tensor.matmul(ps[:], hc8[:, 2 * ks:2 * ks + 2, ts(st, P)],
                                 wv_sb[:, 2 * ks:2 * ks + 2, ts(dc, CH)],
                                 start=(ks == 0), stop=(ks == 3),
                                 perf_mode=DR)
            vdst = Vs[c // 2][:, (c % 2) * 4 + st, dc * 8:dc * 8 + 8, 0:HD]
            nc.vector.scalar_tensor_tensor(
                vdst,
                ps[:].rearrange("p (h d) -> p h d", h=8),
                2.0 ** -K_QKV,
                bvr_sb[:, ts(dc, CH)].rearrange("p (h d) -> p h d", h=8),
                op0=ALU.mult, op1=ALU.add)

        def qproj(qc, hp):
            q8 = q8s[qc]
            ps = mmp.tile([P, CH], F32, name="psq", tag="mm1")
            for ks in range(4):
                nc.tensor.matmul(ps[:], wq_sb[:, 2 * ks:2 * ks + 2, ts(hp, P)],
                                 q8[:, 2 * ks:2 * ks + 2, :],
                                 start=(ks == 0), stop=(ks == 3),
                                 perf_mode=DR)
            nc.vector.tensor_scalar(QTs[qc][:, hp, :], ps[:],
                                    2.0 ** -K_QKV,
                                    bias_sb[:, 16 + hp:17 + hp],
                                    op0=ALU.mult, op1=ALU.add)

        # ---- early dense QKV: chunks kv0, kv1 + qproj 0. vproj first: its
        # st-outer groups consume hc8 one 128-token tile at a time, so the
        # first matmuls start after a single LN tile ----
        make_chunk(0)
        make_chunk(1)
        for st in range(4):
            for dc in range(2):
                vproj(0, st, dc)
        for hp in range(DP):
            kproj(0, hp)
        make_chunk(2)
        for st in range(4):
            for dc in range(2):
                vproj(1, st, dc)
        for hp in range(DP):
            kproj(1, hp)
        for hp in range(DP):
            qproj(0, hp)
        finish_chunk()

        if KPHASES < 1:
            # debug: LN pipeline only; dump hc8 chunks 0..1 as f32
            with tc.tile_pool(name="dbg", bufs=2) as dbg:
                for c in range(2):
                    for i in range(DP):
                        dt_ = dbg.tile([P, CH], F32, name="dt", tag="dt")
                        nc.vector.tensor_copy(dt_[:], hc8s[c][:, i, :])
                        nc.sync.dma_start(out3[:, i, ts(c, CH)], dt_[:])

        # fill queue of leftover phase-1 PE work, pumped between attention
        # m-steps so the PE stays warm while EXP saturates the scalar engine
        fill = deque()

        def pump(n=1):
            for _ in range(n):
                if fill:
                    fill.popleft()()

        if KPHASES >= 1:
            for st in range(2):
                for dc in range(2):
                    fill.append(lambda st=st, dc=dc: vproj(2, st, dc))
            fill.append(lambda: make_chunk(3))
            fill.append(finish_chunk)
            for st in range(2, 4):
                for dc in range(2):
                    fill.append(lambda st=st, dc=dc: vproj(2, st, dc))
            for hp in range(DP):
                fill.append(lambda hp=hp: kproj(2, hp))
            for st in range(4):
                for dc in range(2):
                    fill.append(lambda st=st, dc=dc: vproj(3, st, dc))
            for hp in range(DP):
                fill.append(lambda hp=hp: kproj(3, hp))
            for hp in range(DP):
                fill.append(lambda hp=hp: qproj(1, hp))

        # =================== attention machinery =============================
        scale = float(HD) ** -0.5

        if KPHASES >= 2:
            def scores_kt(t, hp, kt, wei8):
                """Scores + exp for one 128-key tile (both heads of pair hp)."""
                o = 64 * (kt - 8 * t) if kt >= 8 * t else 0
                op = 64 * ((kt & ~1) - 8 * t) if (kt & ~1) >= 8 * t else 0
                KT = KTs[kt // 8]
                ktl = kt % 8
                ps2 = psS.tile([P, 2, QB], F32, name="pss2", tag="pss2")
                for l in range(2):
                    pb = 64 * l
                    nc.tensor.matmul(ps2[:, l, o:],
                                     KT[pb:pb + 64, hp, ts(ktl, P)],
                                     QTs[t][pb:pb + 64, hp, o:],
                                     start=True, stop=True)
                nc.scalar.activation(wei8[:, kt & 1, :, o:], ps2[:, :, o:],
                                     AF.Exp, scale=scale)
                if o > op:
                    # odd diagonal tile: zero the columns its even partner
                    # covers but it does not
                    nc.vector.memset(wei8[:, 1, :, op:o], 0.0)
                if kt >= 8 * t:
                    nc.vector.tensor_mul(wei8[:, kt & 1, :, o:o + 64],
                                         wei8[:, kt & 1, :, o:o + 64],
                                         mask_sb[:])

            def attn_unit(t, hp, per_step=0):
                """One (query block, head pair) attention unit with software
                pipelining: AV for m-1 issues after scores for m, so the AV
                matmuls overlap the exp of the next key pair."""
                nkt = 8 * (t + 1)
                nm = nkt // 2
                pavs = [psAV.tile([P, QB], F32, name=f"pav{l}", tag="pav")
                        for l in range(2)]
                wei8s = {}
                for m in range(nm + 1):
                    if m < nm:
                        wei8s[m] = weip.tile([P, 2, 2, QB], FP8,
                                             name="wei8", tag="wei8")
                        scores_kt(t, hp, 2 * m, wei8s[m])
                        scores_kt(t, hp, 2 * m + 1, wei8s[m])
                    if m >= 1:
                        ma = m - 1
                        op = 64 * (2 * ma - 8 * t) if 2 * ma >= 8 * t else 0
                        wei8 = wei8s.pop(ma)
                        Vx = Vs[ma // 4]
                        mloc = ma % 4
                        for l in range(2):
                            nc.tensor.matmul(pavs[l][0:65, op:],
                                             Vx[:, 2 * mloc:2 * mloc + 2, 2 * hp + l, :],
                                             wei8[:, :, l, op:],
                                             start=(ma == 0), stop=(ma == nm - 1),
                                             perf_mode=DR)
                    pump(per_step)
                # denominator + attn8 write (immediate)
                for l in range(2):
                    pb = 64 * l
                    pav = pavs[l]
                    den = smal.tile([1, QB], F32, name="den", tag="den")
                    nc.vector.tensor_copy(den[:], pav[64:65, :])
                    rec = smal.tile([1, QB], F32, name="rec", tag="rec")
                    nc.vector.reciprocal_approx_fast(rec[:], den[:])
                    rec64 = smal.tile([64, QB], F32, name="rec64", tag="rec64")
                    nc.gpsimd.partition_broadcast(rec64[:], rec[:], channels=64)
                    nc.vector.tensor_mul(attn8s[t][pb:pb + 64, hp, :],
                                         pav[0:64, :], rec64[:])

            # ---- t=0 attention interleaved with the QKV tail ----
            for hp in range(DP):
                attn_unit(0, hp, per_step=1)
            pump(len(fill))  # drain leftovers

        if KPHASES < 2:
            # debug: dump KT (or QT with KDUMP=QT) and stop
            pump(len(fill))
            with tc.tile_pool(name="dbg", bufs=2) as dbg:
                if os.environ.get("KDUMP") == "QT":
                    for i in range(DP):
                        dt_ = dbg.tile([P, QB], F32, name="dt", tag="dt")
                        nc.vector.tensor_copy(dt_[:], QTs[0][:, i, :])
                        nc.sync.dma_start(out3[:, i, 0:QB], dt_[:])
                        dt2 = dbg.tile([P, QB], F32, name="dt2", tag="dt2")
                        nc.vector.tensor_copy(dt2[:], QTs[1][:, i, :])
                        nc.sync.dma_start(out3[:, i, QB:], dt2[:])
                else:
                    for i in range(DP):
                        dt_ = dbg.tile([P, TQ], F32, name="dt", tag="dt")
                        nc.vector.tensor_copy(dt_[:], KTs[0][:, i, :])
                        nc.sync.dma_start(out3[:, i, :], dt_[:])
            p1.close()
            sKVQ.close()
            sATT.close()
            sX2.close()
            sWX.close()

        if KPHASES >= 2:
            # phase-1 pools retire; prefetch w1 (needed mid-t=1 by FFN1 qc0)
            p1.close()
            sPS0.close()
            x2p = sX2.enter_context(tc.tile_pool(name="x2p", bufs=1, side="right"))
            x2Ts.extend(x2p.tile([P, DP, QB], BF16, name=f"x2T{t}") for t in range(2))
            h28s.extend(x2p.tile([P, DP, QB], FP8, name=f"h28_{t}") for t in range(2))
            ffw = ExitStack()
            w1p = ffw.enter_context(tc.tile_pool(name="w1p", bufs=1, side="right"))
            w1_sb = w1p.tile([P, DP, F], FP8, name="w1_sb")
            nc.gpsimd.dma_start(w1_sb[:, :, 0:F // 2], w13[:, :, 0:F // 2])
            nc.gpsimd.dma_start(w1_sb[:, :, F // 2:], w13[:, :, F // 2:])
            rtp = ffw.enter_context(tc.tile_pool(name="rtp", bufs=1, side="right"))
            rT0 = rtp.tile([P, FP, CH], FP8, name="rT", tag="rT")

            psF = sATT.enter_context(tc.tile_pool(name="psF", bufs=2, space="PSUM"))
            sPS1 = ExitStack()   # attention PSUM pools for the t=1 window
            psS = sPS1.enter_context(tc.tile_pool(name="psS2", bufs=2, space="PSUM"))
            psAV = sPS1.enter_context(tc.tile_pool(name="psAV2", bufs=2, space="PSUM"))

            def wo_group(t, i):
                """One out-projection output tile + residual epilogue -> x2T."""
                ps = psF.tile([P, CH], F32, name="psf", tag="psf")
                for ks in range(4):
                    nc.tensor.matmul(ps[:], wo_sb[:, 2 * ks:2 * ks + 2, ts(i, P)],
                                     attn8s[t][:, 2 * ks:2 * ks + 2, :],
                                     start=(ks == 0), stop=(ks == 3),
                                     perf_mode=DR)
                xo = xop.tile([P, CH], F32, name="xo", tag="xo")
                nc.sync.dma_start(xo[:], xoT3[:, i, ts(t, CH)])
                nc.vector.scalar_tensor_tensor(x2Ts[t][:, i, :], ps[:],
                                               2.0 ** -K_O, xo[:],
                                               op0=ALU.mult, op1=ALU.add)

            def ln2_half(qc):
                """LN2 for query half qc -> h28s[qc]. Stats via ones-matmul
                accumulation on the PE; squares on DVE."""
                x2T = x2Ts[qc]
                ps_s1 = psF.tile([P, CH], F32, name="psf", tag="psf")
                for i in range(DP):
                    nc.tensor.matmul(ps_s1[0:1, :], ones_b16[:], x2T[:, i, :],
                                     start=(i == 0), stop=(i == DP - 1))
                negmu2 = l2s.tile([1, CH], F32, name="negmu2", tag="negmu2")
                nc.vector.tensor_scalar_mul(negmu2[:], ps_s1[0:1, :], -1.0 / D)
                ps_s2 = psF.tile([P, CH], F32, name="psf", tag="psf")
                for i in range(DP):
                    sq = l2q.tile([P, CH], BF16, name="sq2", tag="sq2")
                    nc.vector.tensor_mul(sq[:], x2T[:, i, :], x2T[:, i, :])
                    nc.tensor.matmul(ps_s2[0:1, :], ones_b16[:], sq[:],
                                     start=(i == 0), stop=(i == DP - 1))
                muu2 = l2s.tile([1, CH], F32, name="muu2", tag="muu2")
                nc.vector.tensor_mul(muu2[:], negmu2[:], negmu2[:])
                sdb2 = l2s.tile([1, CH], F32, name="sdb2", tag="sdb2")
                nc.vector.tensor_scalar(sdb2[:], muu2[:], -1.0, EPS,
                                        op0=ALU.mult, op1=ALU.add)
                var2 = l2s.tile([1, CH], F32, name="var2", tag="var2")
                nc.vector.scalar_tensor_tensor(var2[:], ps_s2[0:1, :],
                                               1.0 / D, sdb2[:],
                                               op0=ALU.mult, op1=ALU.add)
                sd2 = l2s.tile([1, CH], F32, name="sd2", tag="sd2")
                nc.scalar.activation(sd2[:], var2[:], AF.Sqrt)
                rstd2 = l2s.tile([1, CH], F32, name="rstd2", tag="rstd2")
                nc.vector.reciprocal_approx_fast(rstd2[:], sd2[:])
                nmr2 = l2s.tile([1, CH], BF16, name="nmr2", tag="nmr2")
                nc.vector.tensor_mul(nmr2[:], negmu2[:], rstd2[:])
                rstd2b = l2s.tile([P, CH], F32, name="rstd2b", tag="rstd2b")
                nc.gpsimd.partition_broadcast(rstd2b[:], rstd2[:], channels=P)
                nmr2b = l2s.tile([P, CH], BF16, name="nmr2b", tag="nmr2b")
                nc.gpsimd.partition_broadcast(nmr2b[:], nmr2[:], channels=P)
                for i in range(DP):
                    tmp = l2q.tile([P, CH], BF16, name="h2t", tag="h2t")
                    nc.vector.tensor_mul(tmp[:], x2Ts[qc][:, i, :], rstd2b[:])
                    nc.vector.tensor_add(h28s[qc][:, i, :], tmp[:], nmr2b[:])

            def ffn1_group(qc, f, rT):
                h28 = h28s[qc]
                ps = psF.tile([P, CH], F32, name="psf", tag="psf")
                for ks in range(4):
                    nc.tensor.matmul(ps[:], w1_sb[:, 2 * ks:2 * ks + 2, ts(f, P)],
                                     h28[:, 2 * ks:2 * ks + 2, :],
                                     start=(ks == 0), stop=(ks == 3),
                                     perf_mode=DR)
                nc.scalar.activation(rT[:, f, :], ps[:],
                                     AF.Relu,
                                     bias=bias_sb[:, 32 + f:33 + f],
                                     scale=2.0 ** -K_1)

            # ---- t=1 attention interleaved with wo(0), LN2(0), FFN1 qc0.
            # Spacers give the serial LN2 chain ~2 attention units of slack
            # before FFN1 enters the in-order PE queue (else the PE and the
            # EXP stream stall behind a not-yet-ready FFN1 group). ----
            for i in range(DP):
                fill.append(lambda i=i: wo_group(0, i))
            fill.append(lambda: ln2_half(0))
            for _ in range(9):
                fill.append(lambda: None)
            for f in range(FP):
                fill.append(lambda f=f: ffn1_group(0, f, rT0))

            for hp in range(DP):
                attn_unit(1, hp, per_step=(2 if hp == 0 else 0 if hp == 1 else 1))
            pump(len(fill))
            sKVQ.close()
            sPS1.close()

        if KPHASES == 2:
            # debug: dump x2T (or attn8 with KDUMP=ATT) and stop
            with tc.tile_pool(name="dbg", bufs=2) as dbg:
                for t in range(2):
                    dsrc = attn8s[t] if os.environ.get("KDUMP") == "ATT" else x2Ts[t]
                    for i in range(DP):
                        dt_ = dbg.tile([P, QB], F32, name="dt", tag="dt")
                        nc.vector.tensor_copy(dt_[:], dsrc[:, i, :])
                        nc.sync.dma_start(out3[:, i, ts(t, CH)], dt_[:])
            ffw.close()
            sX2.close()
            sATT.close()
            sWX.close()

        if KPHASES >= 3:
            # ============== tail: wo(1), LN2(1), FFN2 qc0, FFN1/2 qc1 =======
            # attention PSUM pools have retired so FFN2 gets banks; w2 now
            # w2 in two half tiles so FFN2 output tiles 0-3 only wait on the
            # first 2 MB of the prefetch
            w2p = ffw.enter_context(tc.tile_pool(name="w2p", bufs=1, side="right"))
            w2_sbs = [w2p.tile([P, FP, D // 2], FP8, name=f"w2_sb{h}")
                      for h in range(2)]
            nc.gpsimd.dma_start(w2_sbs[0][:], w23[:, :, 0:D // 2])
            nc.gpsimd.dma_start(w2_sbs[1][:], w23[:, :, D // 2:])

            with tc.tile_pool(name="psO", bufs=2, space="PSUM") as psO, \
                 tc.tile_pool(name="top", bufs=4) as top:

                def ffn2_group(qc, i, rT):
                    w2_sb = w2_sbs[i // 4]
                    io = i % 4
                    ps2 = psO.tile([P, CH], F32, name="ps2", tag="ps2")
                    for f2 in range(FP // 2):
                        nc.tensor.matmul(ps2[:],
                                         w2_sb[:, 2 * f2:2 * f2 + 2, ts(io, P)],
                                         rT[:, 2 * f2:2 * f2 + 2, :],
                                         start=(f2 == 0), stop=False,
                                         perf_mode=DR)
                    # + b2 via a K=1 ones-matmul (b2r pre-scaled by 2**K_2)
                    nc.tensor.matmul(ps2[:], b2r_sb[0:1, ts(i, P)], onesq[:],
                                     start=False, stop=True)
                    ot = top.tile([P, CH], F32, name="ot", tag="ot")
                    nc.vector.scalar_tensor_tensor(
                        ot[:], ps2[:], 2.0 ** -K_2,
                        x2Ts[qc][:, i, :], op0=ALU.mult, op1=ALU.add)
                    nc.sync.dma_start(out3[:, i, ts(qc, CH)], ot[:])

                for i in range(DP):
                    wo_group(1, i)
                ln2_half(1)
                # FFN2 qc0 overlaps LN2(1)'s vector work; then the single rT
                # slot rotates to qc1 (alloc waits on the qc0 reads)
                for i in range(DP):
                    ffn2_group(0, i, rT0)
                rT1 = rtp.tile([P, FP, CH], FP8, name="rT", tag="rT")
                for f in range(FP):
                    ffn1_group(1, f, rT1)
                for i in range(DP):
                    ffn2_group(1, i, rT1)
            ffw.close()
            sX2.close()
            sATT.close()
            sWX.close()

    nc.compile()
    return nc


def _fp8_scale(w, k):
    """Scale w by 2**k and cast to fp8e4m3 (max-normal 240), asserting range."""
    s = np.asarray(w, np.float32) * (2.0 ** k)
    assert np.abs(s).max() < 239.0, f"fp8 overflow: {np.abs(s).max()}"
    return np.ascontiguousarray(s.astype(ml_dtypes.float8_e4m3))


def prepare_inputs(x, wq, wk, wv, wo, bo, w1, b1, w2, b2,
                   g_ln1, b_ln1, g_ln2, b_ln2):
    """Host-side sharding/prep. Returns list of 8 per-core input dicts."""
    f32 = np.float32
    bf = ml_dtypes.bfloat16
    x = np.asarray(x, f32)
    g1 = np.asarray(g_ln1, f32)
    b1n = np.asarray(b_ln1, f32)
    g2 = np.asarray(g_ln2, f32)
    b2n = np.asarray(b_ln2, f32)

    wq_e = _fp8_scale(g1[:, None] * np.asarray(wq, f32), K_QKV)
    wk_e = _fp8_scale(g1[:, None] * np.asarray(wk, f32), K_QKV)
    wv_e = _fp8_scale(g1[:, None] * np.asarray(wv, f32), K_QKV)
    wo_e = _fp8_scale(np.asarray(wo, f32), K_O)
    w1_e = _fp8_scale(g2[:, None] * np.asarray(w1, f32), K_1)
    w2_e = _fp8_scale(np.asarray(w2, f32), K_2)

    bq = b1n @ np.asarray(wq, f32)
    bk = b1n @ np.asarray(wk, f32)
    bv = b1n @ np.asarray(wv, f32)
    b1p = np.asarray(b1, f32) + b2n @ np.asarray(w1, f32)
    bo_f = np.asarray(bo, f32)
    b2_f = np.asarray(b2, f32)

    def pcol(v, n):  # [n*128] -> [128, n] partition-major
        return np.ascontiguousarray(np.asarray(v, f32).reshape(n, P).T)

    biases = np.zeros((P, 64), f32)
    biases[:, 8:16] = pcol(b2_f, 8)
    biases[:, 16:24] = pcol(bq, 8)
    biases[:, 24:32] = pcol(bk, 8)
    biases[:, 32:64] = pcol(b1p, 32)
    bvr = np.ascontiguousarray(np.broadcast_to(bv[None, :], (P, D)))

    # per-parity token permutation: queries always land on even positions
    perms = {}
    for d in (0, 1):
        perm = np.arange(TKV)
        if d == 1:
            perm = perm.reshape(-1, 2)[:, ::-1].reshape(-1)
        perms[d] = perm

    # diagonal-tile causal masks: key at tile position p (original token
    # perm[base+p]) may be attended by query column r (original token
    # perm[base+2r] = base + 2r + d) iff perm[base+p] <= base + 2r + d.
    masks = {}
    for d in (0, 1):
        pp = perms[d][:P]               # relative original positions
        r = np.arange(64)[None, :]
        m = (pp[:, None] <= (2 * r + d)).astype(bf)
        masks[d] = np.ascontiguousarray(np.broadcast_to(m[:, None, :], (P, 2, 64)))

    in_maps = []
    for c in range(8):
        b, d = divmod(c, 2)
        xo = x[b, d::2].T + bo_f[:, None]
        in_maps.append(dict(
            xn=np.ascontiguousarray(x[b][perms[d]].astype(bf)),
            xoT=np.ascontiguousarray(xo),
            wq=wq_e, wk=wk_e, wv=wv_e, wo=wo_e, w1=w1_e, w2=w2_e,
            biases=biases, bvr=bvr, mk=masks[d],
            b2r=np.ascontiguousarray((b2_f * 2.0 ** K_2)[None, :].astype(bf)),
        ))
    return in_maps


_NC = None
LAST_RESULTS = None


def kernel(**inputs):
    global _NC, LAST_RESULTS
    in_maps = prepare_inputs(**inputs)
    if _NC is None:
        _NC = build_nc()
    res = run_bass_kernel_spmd(_NC, in_maps, core_ids=list(range(8)))
    LAST_RESULTS = res
    out = np.empty((4, TKV, D), np.float32)
    for c in range(8):
        b, d = divmod(c, 2)
        out[b, d::2, :] = res.results[c]["outT"].T
    return out


if __name__ == "__main__":
    z = np.load("/root/problem/ref_cache.npz")
    inputs = {k: z[k] for k in z.files if k != "out"}
    out = kernel(**inputs)
    ref = z["out"]
    err = np.abs(out - ref)
    print("abs max err:", err.max(), "scale-rel:", err.max() / np.abs(ref).max())
